# revision 7
# baseline (speedup 1.0000x reference)
"""Trainium2 Bass kernel for nn_CNN_tagger (multi-width 1D conv + linear tagger).

Strategy: data-parallel over batch across 8 NeuronCores (4 batches/core,
weights replicated). The conv contraction runs on the PE in fp8 DoubleRow
mode (2 contraction planes per matmul, 0.5 cycles/output-row = 4x the bf16
MAC rate). Precision is recovered with a two-sided hi-lo split, all scales
exact powers of two so every plane accumulates into the same PSUM with no
rescaling:
    w·x ~= w8·x8 + wlo·x8 + w8·xlo
      w8 = e4m3(w·8), wlo = e5m2(w·8 - w8)   (e5m2's wide exponent range
      x8 = e4m3(x/8), xlo = e5m2(x/8 - x8)    holds the small residuals)
Mixed-dtype DoubleRow operands (e5m2 x e4m3) are verified on HW, so the
corrections reuse the main operands: 3 matmuls per (tap, 256-ch block)
instead of 4 bf16-equivalents -> conv PE time 0.75x bf16 at rel err ~2e-3
(measured exactly offline; inputs are seeded). Per (batch, branch, 128-out
tile): k taps x 2 blocks x 3 planes DoubleRow matmuls accumulate in PSUM,
bias+relu fused on DVE into fp16 feats, final linear as 6 fp16 matmuls.
Four parallel DMA queues (w8/sync, wlo/gpsimd, x8/scalar, xlo/vector) keep
the prologue ~1.5us; weights stream branch-ordered (k3 first) behind tiny
biases; PE warmup matmuls hold the clock-gate at 2.4GHz.
Host side: quantize + rearrange inputs, transpose the [B, NOUT, S] device
output back to [B, S, NOUT].
"""

import sys

sys.path.insert(0, "/opt/trn_rl_repo")

import ml_dtypes
import numpy as np

import concourse.tile as tile
import concourse.mybir as mybir
from concourse import bacc
from concourse.bass_utils import run_bass_kernel_spmd

B, S, D = 32, 512, 512
NK = 256
KS = (3, 5, 7)
NOUT = 64
NCORES = 8
BPC = B // NCORES  # batches per core
PAD = 3  # (max(KS) - 1) // 2, baked into the padded x layout
SP = S + 2 * PAD
NTAP = sum(KS)  # 15
CIN = NK * len(KS)  # 768
NCT = CIN // 128  # 6 channel tiles
F32 = mybir.dt.float32
F16 = mybir.dt.float16
E4 = mybir.dt.float8e4
E5 = mybir.dt.float8e5
DR = mybir.MatmulPerfMode.DoubleRow
nE4 = ml_dtypes.float8_e4m3
nE5 = ml_dtypes.float8_e5m2

WSCALE = 8.0  # w stored as e4m3(w*8), x as e4m3(x/8); product scale folds to 1

# conv plane phases emitted per (branch, local tap): main + w-correction +
# x-correction. Dropping entries trades measured rel-err for PE time
# (each dropped (tap, side) saves ~1.7us/core and adds ~4.7e-5 err variance).
def _phases_for(k, t):
    return ("m", "cw", "cx")

# (global tap index, x-offset within padded row) per branch
_TAPS = []
_g = 0
for _k in KS:
    _pk = (_k - 1) // 2
    _TAPS.append([(_g + _t, PAD - _pk + _t) for _t in range(_k)])
    _g += _k


def _build(reps=1):
    nc = bacc.Bacc("TRN2")
    x8 = nc.dram_tensor("x8", [BPC, 2, 128, 2, SP], E4, kind="ExternalInput").ap()
    xlo = nc.dram_tensor("xlo", [BPC, 2, 128, 2, SP], E5, kind="ExternalInput").ap()
    w8 = nc.dram_tensor("w8", [2, 128, 2, NTAP, NK], E4, kind="ExternalInput").ap()
    wlo = nc.dram_tensor("wlo", [2, 128, 2, NTAP, NK], E5, kind="ExternalInput").ap()
    lw = nc.dram_tensor("lw", [CIN, NOUT], F16, kind="ExternalInput").ap()
    cb = nc.dram_tensor("cb", [128, NCT], F32, kind="ExternalInput").ap()
    lb = nc.dram_tensor("lb", [128, 1], F32, kind="ExternalInput").ap()
    out = nc.dram_tensor("o", [BPC, NOUT, S], F32, kind="ExternalOutput").ap()

    with tile.TileContext(nc) as tc:
        with (
            tc.tile_pool(name="wpool", bufs=1) as wpool,
            tc.tile_pool(name="cpool", bufs=1) as cpool,
            tc.tile_pool(name="xpool", bufs=1) as xpool,
            tc.tile_pool(name="fpool", bufs=1) as fpool,
            tc.tile_pool(name="pspool", bufs=4, space="PSUM") as pspool,
            tc.tile_pool(name="lpspool", bufs=2, space="PSUM") as lpspool,
            tc.tile_pool(name="opool", bufs=2) as opool,
        ):
            w8_sb = [wpool.tile([128, 2, NTAP, NK], E4, name=f"w8_{k}") for k in range(2)]
            wlo_sb = [wpool.tile([128, 2, NTAP, NK], E5, name=f"wlo_{k}") for k in range(2)]
            lw_sb = [cpool.tile([128, NOUT], F16, name=f"lw_{j}") for j in range(NCT)]
            cb_sb = cpool.tile([128, NCT], F32, name="cb")
            lb_sb = cpool.tile([128, 1], F32, name="lb")
            x8_sb = {}
            xlo_sb = {}
            for b in range(BPC):
                for blk in range(2):
                    x8_sb[(b, blk)] = xpool.tile([128, 2, SP], E4, name=f"x8_b{b}_k{blk}")
                    xlo_sb[(b, blk)] = xpool.tile([128, 2, SP], E5, name=f"xlo_b{b}_k{blk}")

            # tiny bias tensors first on the ACT queue
            nc.scalar.dma_start(cb_sb[:], cb[:, :])
            nc.scalar.dma_start(lb_sb[:], lb[:, :])

            # Three DMA streams ordered by first-use time. The first (k3, b0)
            # group consumes w8k3b0, x8b0b0, wlok3b0, xlob0b0, then the blk1
            # set; SP interleaves w8/wlo for k3 so both arrive in step, ACT
            # interleaves b0's x8/xlo, and gpsimd (slow ~1us/chunk SWDGE)
            # carries everything needed later than ~5us.
            k3 = KS[0]
            for blk in range(2):
                nc.sync.dma_start(
                    w8_sb[blk][:, :, 0:k3, :], w8[blk, :, :, 0:k3, :]
                )
                nc.sync.dma_start(
                    wlo_sb[blk][:, :, 0:k3, :], wlo[blk, :, :, 0:k3, :]
                )
                nc.scalar.dma_start(x8_sb[(0, blk)][:], x8[0, blk])
                nc.scalar.dma_start(xlo_sb[(0, blk)][:], xlo[0, blk])
            # gpsimd: later batches' xlo, then k5/k7 wlo
            for b in range(1, BPC):
                for blk in range(2):
                    nc.gpsimd.dma_start(xlo_sb[(b, blk)][:], xlo[b, blk])
            t0 = k3
            for k in KS[1:]:
                for blk in range(2):
                    nc.sync.dma_start(
                        w8_sb[blk][:, :, t0 : t0 + k, :], w8[blk, :, :, t0 : t0 + k, :]
                    )
                    nc.gpsimd.dma_start(
                        wlo_sb[blk][:, :, t0 : t0 + k, :],
                        wlo[blk, :, :, t0 : t0 + k, :],
                    )
                t0 += k
            for b in range(1, BPC):
                for blk in range(2):
                    nc.scalar.dma_start(x8_sb[(b, blk)][:], x8[b, blk])
            for j in range(NCT):
                nc.sync.dma_start(lw_sb[j][:], lw[j * 128 : (j + 1) * 128, :])

            # PE warmup on a zeroed fp8 tile during the DMA prologue: keeps
            # the clock-gate warm so the real stream starts at 2.4GHz
            wm = cpool.tile([128, 2, 256], E4, name="wm")
            nc.vector.memset(wm[:].bitcast(F32), 0.0)
            for g in range(2):
                wps = pspool.tile([128, 256], F32, tag="ps", name=f"warm_ps_{g}")
                for i in range(8):
                    nc.tensor.matmul(
                        wps[:],
                        wm[:, :, 0:128],
                        wm[:, :, :],
                        start=(i == 0),
                        stop=(i == 7),
                        perf_mode=DR,
                    )

            def one_workload(rep):
                feats = {}

                def linear(b):
                    lps = lpspool.tile([NOUT, S], F32, tag="lps", name=f"lps_r{rep}_b{b}")
                    for j in range(NCT):
                        nc.tensor.matmul(
                            lps[:],
                            lw_sb[j][:],
                            feats[(b, j)][:],
                            start=(j == 0),
                            stop=(j == NCT - 1),
                        )
                    osb = opool.tile([NOUT, S], F32, tag="osb", name=f"o_r{rep}_b{b}")
                    nc.vector.tensor_scalar_add(osb[:], lps[:], lb_sb[0:NOUT, 0:1])
                    nc.scalar.dma_start(out[b], osb[:])

                # branch-outer / batch-inner: all batches' k3 groups first, so
                # early compute covers the k5/k7 weight stream
                for br, taps in enumerate(_TAPS):
                    k = KS[br]
                    for b in range(BPC):
                        for ct in range(2):
                            j = br * 2 + ct
                            cs = slice(ct * 128, (ct + 1) * 128)
                            ps = pspool.tile(
                                [128, S], F32, tag="ps", name=f"ps_r{rep}_b{b}_j{j}"
                            )
                            sched = [
                                (ph, tap, off)
                                for blk_ in (0,)
                                for tl, (tap, off) in enumerate(taps)
                                for ph in _phases_for(k, tl)
                            ]
                            nmm = 2 * len(sched)
                            i = 0
                            for blk in range(2):
                                for tl, (tap, off) in enumerate(taps):
                                    for ph in _phases_for(k, tl):
                                        if ph == "m":
                                            lhs = w8_sb[blk][:, :, tap, cs]
                                            rhs = x8_sb[(b, blk)][:, :, off : off + S]
                                        elif ph == "cw":
                                            lhs = wlo_sb[blk][:, :, tap, cs]
                                            rhs = x8_sb[(b, blk)][:, :, off : off + S]
                                        else:  # "cx"
                                            lhs = w8_sb[blk][:, :, tap, cs]
                                            rhs = xlo_sb[(b, blk)][:, :, off : off + S]
                                        nc.tensor.matmul(
                                            ps[:],
                                            lhs,
                                            rhs,
                                            start=(i == 0),
                                            stop=(i == nmm - 1),
                                            perf_mode=DR,
                                        )
                                        i += 1
                            f = fpool.tile(
                                [128, S], F16, name=f"f_r{rep}_b{b}_j{j}", tag=f"f_b{b}_j{j}"
                            )
                            # f = max(ps + bias, 0), rounded to fp16
                            nc.vector.tensor_scalar(
                                f[:],
                                ps[:],
                                cb_sb[:, j : j + 1],
                                0.0,
                                mybir.AluOpType.add,
                                mybir.AluOpType.max,
                            )
                            feats[(b, j)] = f
                            if br == len(_TAPS) - 1 and ct == 1:
                                linear(b)

            if reps == 1:
                one_workload(0)
            else:
                with tc.For_i(0, reps, 1, hint_engines=(mybir.EngineType.PE,)):
                    one_workload(0)

    nc.compile()
    return nc


def _prep_inputs(x, conv_w3, conv_b3, conv_w5, conv_b5, conv_w7, conv_b7, lin_w, lin_b):
    x = np.asarray(x, np.float32)
    xs = np.zeros((B, D, SP), np.float32)
    xs[:, :, PAD : PAD + S] = x.transpose(0, 2, 1) * np.float32(1.0 / WSCALE)
    x8 = xs.astype(nE4)
    xlo = (xs - x8.astype(np.float32)).astype(nE5)
    # [B, D, SP] -> [B, blk, plane, part, SP] -> [B, blk, part, plane, SP]
    def rearr_x(a):
        return np.ascontiguousarray(
            a.reshape(B, 2, 2, 128, SP).transpose(0, 1, 3, 2, 4)
        )
    x8r, xlor = rearr_x(x8), rearr_x(xlo)

    # W[d, tap, c] = conv_wk[c, d, t] * 8, taps stacked k3|k5|k7
    W = np.concatenate(
        [
            np.asarray(cw, np.float32).transpose(1, 2, 0)
            for cw in (conv_w3, conv_w5, conv_w7)
        ],
        axis=1,
    ) * np.float32(WSCALE)
    w8 = W.astype(nE4)
    wlo = (W - w8.astype(np.float32)).astype(nE5)
    def rearr_w(a):
        return np.ascontiguousarray(
            a.reshape(2, 2, 128, NTAP, NK).transpose(0, 2, 1, 3, 4)
        )
    w8r, wlor = rearr_w(w8), rearr_w(wlo)

    lwT = np.ascontiguousarray(np.asarray(lin_w, np.float32).T.astype(np.float16))
    cbT = np.ascontiguousarray(
        np.concatenate(
            [np.asarray(b_, np.float32) for b_ in (conv_b3, conv_b5, conv_b7)]
        ).reshape(NCT, 128).T
    )
    lb1 = np.asarray(lin_b, np.float32).reshape(NOUT, 1)
    lb2 = np.ascontiguousarray(np.concatenate([lb1, lb1], axis=0))
    return [
        {
            "x8": np.ascontiguousarray(x8r[c * BPC : (c + 1) * BPC]),
            "xlo": np.ascontiguousarray(xlor[c * BPC : (c + 1) * BPC]),
            "w8": w8r,
            "wlo": wlor,
            "lw": lwT,
            "cb": cbT,
            "lb": lb2,
        }
        for c in range(NCORES)
    ]


_NC_CACHE = {}


def _get_nc(reps=1):
    if reps not in _NC_CACHE:
        _NC_CACHE[reps] = _build(reps)
    return _NC_CACHE[reps]


def kernel(x, conv_w3, conv_b3, conv_w5, conv_b5, conv_w7, conv_b7, lin_w, lin_b):
    nc = _get_nc(1)
    in_maps = _prep_inputs(
        x, conv_w3, conv_b3, conv_w5, conv_b5, conv_w7, conv_b7, lin_w, lin_b
    )
    res = run_bass_kernel_spmd(nc, in_maps, list(range(NCORES)))
    outT = np.concatenate([res.results[c]["o"] for c in range(NCORES)], axis=0)
    return np.ascontiguousarray(outT.transpose(0, 2, 1))


# revision 9
# speedup vs baseline: 1.4258x; 1.4258x over previous
"""Trainium2 Bass kernel for nn_CNN_tagger (multi-width 1D conv + linear tagger).

Strategy: data-parallel over batch across 8 NeuronCores (4 batches/core,
conv + linear weights replicated). Per batch, each conv branch k in {3,5,7}
is computed as k shifted [D=512]-contraction matmuls accumulated in PSUM
(15 taps x 4 d-tiles = 60 matmuls per 128-channel tile), bias+relu fused on
the vector/pool engines (alternating, so neither serializes the PE), and the
final linear layer as 6 accumulated fp16 matmuls. All conv matmuls run in
bf16 (PE rate identical to fp32r at free-dim 512 — 1 elem/cell/cycle — but
half the DMA traffic and SBUF footprint; measured end-to-end rel err 3e-3).
fp8 DoubleRow was evaluated and rejected: on real TRN2 it only reaches
~1.44x bf16 (LDWEIGHTS for 256 columns disables FWL), which the required
precision-restoring correction matmuls more than consume.
Six PSUM banks rotate for conv groups (+2 for the linear) so the PE can run
several accumulation groups ahead of the relu drain. Work is ordered
branch-outer/batch-inner with the weight stream branch-ordered behind the
biases so compute covers the DMA prologue, plus PE warmup matmuls to hold
the HAM clock-gate at 2.4GHz.
Host side: transpose x to [B, D, S] bf16 with zero padding baked in,
pre-arrange weights as [D, tap, c] bf16, linear weights fp16, and transpose
the [B, NOUT, S] device output back to [B, S, NOUT].
"""

import sys

sys.path.insert(0, "/opt/trn_rl_repo")

import ml_dtypes
import numpy as np

import concourse.tile as tile
import concourse.mybir as mybir
from concourse import bacc
from concourse.bass_utils import run_bass_kernel_spmd

B, S, D = 32, 512, 512
NK = 256
KS = (3, 5, 7)
NOUT = 64
NCORES = 8
BPC = B // NCORES  # batches per core
PAD = 3  # (max(KS) - 1) // 2, baked into the padded x layout
SP = S + 2 * PAD
NTAP = sum(KS)  # 15
CIN = NK * len(KS)  # 768
NCT = CIN // 128  # 6 channel tiles
F32 = mybir.dt.float32
F16 = mybir.dt.float16
BF = mybir.dt.bfloat16

# (global tap index, x-offset within padded row) per branch
_TAPS = []
_g = 0
for _k in KS:
    _pk = (_k - 1) // 2
    _TAPS.append([(_g + _t, PAD - _pk + _t) for _t in range(_k)])
    _g += _k


def _build(reps=1):
    nc = bacc.Bacc("TRN2")
    x = nc.dram_tensor("x", [BPC, D, SP], BF, kind="ExternalInput").ap()
    w = nc.dram_tensor("w", [D, NTAP, NK], BF, kind="ExternalInput").ap()
    lw = nc.dram_tensor("lw", [CIN, NOUT], F16, kind="ExternalInput").ap()
    cb = nc.dram_tensor("cb", [128, NCT], F32, kind="ExternalInput").ap()
    lb = nc.dram_tensor("lb", [128, 1], F32, kind="ExternalInput").ap()
    out = nc.dram_tensor("o", [BPC, NOUT, S], F32, kind="ExternalOutput").ap()

    with tile.TileContext(nc) as tc:
        with (
            tc.tile_pool(name="wpool", bufs=1) as wpool,
            tc.tile_pool(name="cpool", bufs=1) as cpool,
            tc.tile_pool(name="xpool", bufs=1) as xpool,
            tc.tile_pool(name="fpool", bufs=1) as fpool,
            tc.tile_pool(name="pspool", bufs=6, space="PSUM") as pspool,
            tc.tile_pool(name="lpspool", bufs=2, space="PSUM") as lpspool,
            tc.tile_pool(name="opool", bufs=2) as opool,
        ):
            w_sb = [
                wpool.tile([128, NTAP, NK], BF, name=f"w_{d}") for d in range(4)
            ]
            lw_sb = [
                cpool.tile([128, NOUT], F16, name=f"lw_{j}") for j in range(NCT)
            ]
            cb_sb = cpool.tile([128, NCT], F32, name="cb")
            lb_sb = cpool.tile([128, 1], F32, name="lb")

            xb = {}
            for b in range(BPC):
                for d in range(4):
                    xb[(b, d)] = xpool.tile([128, SP], BF, name=f"x_b{b}_d{d}")

            # tiny bias tensors first: every relu needs cb
            nc.scalar.dma_start(cb_sb[:], cb[:, :])
            nc.scalar.dma_start(lb_sb[:], lb[:, :])
            # batch-0 x leads on the ACT queue; weights stream branch-ordered
            # (k3 first) on the SP queue so the first groups start ~1us in
            for d in range(4):
                nc.scalar.dma_start(xb[(0, d)][:], x[0, d * 128 : (d + 1) * 128, :])
            t0 = 0
            for ki, k in enumerate(KS):
                for d in range(4):
                    nc.sync.dma_start(
                        w_sb[d][:, t0 : t0 + k, :],
                        w[d * 128 : (d + 1) * 128, t0 : t0 + k, :],
                    )
                t0 += k
                if ki == 0:
                    for b in range(1, BPC):
                        for d in range(4):
                            nc.scalar.dma_start(
                                xb[(b, d)][:], x[b, d * 128 : (d + 1) * 128, :]
                            )
            for j in range(NCT):
                nc.sync.dma_start(lw_sb[j][:], lw[j * 128 : (j + 1) * 128, :])

            # PE warmup: dummy matmuls on a zeroed tile during the DMA
            # prologue keep the HAM clock-gate warm
            wm = cpool.tile([128, 640], BF, name="wm")
            nc.vector.memset(wm[:].bitcast(F32), 0.0)
            for g in range(2):
                wps = pspool.tile([128, S], F32, tag="ps", name=f"warm_ps_{g}")
                for i in range(4):
                    nc.tensor.matmul(
                        wps[:],
                        wm[:, 0:128],
                        wm[:, 128:640],
                        start=(i == 0),
                        stop=(i == 3),
                    )

            def one_workload(rep):
                feats = {}

                def linear(b):
                    lps = lpspool.tile([NOUT, S], F32, tag="lps", name=f"lps_r{rep}_b{b}")
                    for j in range(NCT):
                        nc.tensor.matmul(
                            lps[:],
                            lw_sb[j][:],
                            feats[(b, j)][:],
                            start=(j == 0),
                            stop=(j == NCT - 1),
                        )
                    osb = opool.tile([NOUT, S], F32, tag="osb", name=f"o_r{rep}_b{b}")
                    nc.vector.tensor_scalar_add(osb[:], lps[:], lb_sb[0:NOUT, 0:1])
                    nc.scalar.dma_start(out[b], osb[:])

                # branch-outer / batch-inner: all batches' k3 groups first, so
                # early compute covers the k5/k7 weight stream; each batch's
                # linear layer runs right after its last k7 group
                gi = 0
                for br, taps in enumerate(_TAPS):
                    for b in range(BPC):
                        for ct in range(2):
                            j = br * 2 + ct
                            ps = pspool.tile(
                                [128, S], F32, tag="ps", name=f"ps_r{rep}_b{b}_j{j}"
                            )
                            nmm = len(taps) * 4
                            i = 0
                            # d-outer: each arriving weight d-chunk unlocks a
                            # run of taps, smoothing the prologue dribble
                            for d in range(4):
                                for tap, off in taps:
                                    nc.tensor.matmul(
                                        ps[:],
                                        w_sb[d][:, tap, ct * 128 : (ct + 1) * 128],
                                        xb[(b, d)][:, off : off + S],
                                        start=(i == 0),
                                        stop=(i == nmm - 1),
                                    )
                                    i += 1
                            f = fpool.tile(
                                [128, S], F16, name=f"f_r{rep}_b{b}_j{j}", tag=f"f_b{b}_j{j}"
                            )
                            # f = max(ps + bias, 0) in fp16; alternate DVE and
                            # ACT so the PSUM drain never gates the PE
                            if gi % 2 == 0:
                                nc.vector.tensor_scalar(
                                    f[:],
                                    ps[:],
                                    cb_sb[:, j : j + 1],
                                    0.0,
                                    mybir.AluOpType.add,
                                    mybir.AluOpType.max,
                                )
                            else:
                                nc.scalar.activation(
                                    f[:],
                                    ps[:],
                                    mybir.ActivationFunctionType.Relu,
                                    bias=cb_sb[:, j : j + 1],
                                )
                            gi += 1
                            feats[(b, j)] = f
                            if br == len(_TAPS) - 1 and ct == 1:
                                linear(b)

            if reps == 1:
                one_workload(0)
            else:
                with tc.For_i(0, reps, 1, hint_engines=(mybir.EngineType.PE,)):
                    one_workload(0)

    nc.compile()
    return nc


def _prep_inputs(x, conv_w3, conv_b3, conv_w5, conv_b5, conv_w7, conv_b7, lin_w, lin_b):
    x = np.asarray(x, np.float32)
    xp = np.zeros((B, D, SP), ml_dtypes.bfloat16)
    xp[:, :, PAD : PAD + S] = x.transpose(0, 2, 1).astype(ml_dtypes.bfloat16)
    # W[d, tap, c] = conv_wk[c, d, t], taps stacked k3|k5|k7
    W = np.ascontiguousarray(
        np.concatenate(
            [
                np.asarray(cw, np.float32).transpose(1, 2, 0)
                for cw in (conv_w3, conv_w5, conv_w7)
            ],
            axis=1,
        ).astype(ml_dtypes.bfloat16)
    )
    lwT = np.ascontiguousarray(np.asarray(lin_w, np.float32).T.astype(np.float16))
    cbT = np.ascontiguousarray(
        np.concatenate(
            [np.asarray(b_, np.float32) for b_ in (conv_b3, conv_b5, conv_b7)]
        ).reshape(NCT, 128).T
    )
    lb1 = np.asarray(lin_b, np.float32).reshape(NOUT, 1)
    lb2 = np.ascontiguousarray(np.concatenate([lb1, lb1], axis=0))
    return [
        {
            "x": np.ascontiguousarray(xp[c * BPC : (c + 1) * BPC]),
            "w": W,
            "lw": lwT,
            "cb": cbT,
            "lb": lb2,
        }
        for c in range(NCORES)
    ]


_NC_CACHE = {}


def _get_nc(reps=1):
    if reps not in _NC_CACHE:
        _NC_CACHE[reps] = _build(reps)
    return _NC_CACHE[reps]


def kernel(x, conv_w3, conv_b3, conv_w5, conv_b5, conv_w7, conv_b7, lin_w, lin_b):
    nc = _get_nc(1)
    in_maps = _prep_inputs(
        x, conv_w3, conv_b3, conv_w5, conv_b5, conv_w7, conv_b7, lin_w, lin_b
    )
    res = run_bass_kernel_spmd(nc, in_maps, list(range(NCORES)))
    outT = np.concatenate([res.results[c]["o"] for c in range(NCORES)], axis=0)
    return np.ascontiguousarray(outT.transpose(0, 2, 1))


# revision 16
# speedup vs baseline: 1.5118x; 1.0603x over previous
"""Trainium2 Bass kernel for nn_CNN_tagger (multi-width 1D conv + linear tagger).

Strategy: data-parallel over batch across 8 NeuronCores (4 batches/core,
conv + linear weights replicated). Per batch, each conv branch k in {3,5,7}
is computed as k shifted [D=512]-contraction matmuls accumulated in PSUM
(15 taps x 4 d-tiles = 60 matmuls per 128-channel tile), bias+relu fused on
the vector/pool engines (alternating, so neither serializes the PE), and the
final linear layer as 6 accumulated fp16 matmuls. All conv matmuls run in
bf16 (PE rate identical to fp32r at free-dim 512 — 1 elem/cell/cycle — but
half the DMA traffic and SBUF footprint; measured end-to-end rel err 3e-3).
fp8 DoubleRow was evaluated and rejected: on real TRN2 it only reaches
~1.44x bf16 (LDWEIGHTS for 256 columns disables FWL), which the required
precision-restoring correction matmuls more than consume.
Six PSUM banks rotate for conv groups (+2 for the linear) so the PE can run
several accumulation groups ahead of the relu drain. Work is ordered
branch-outer/batch-inner with the weight stream branch-ordered behind the
biases so compute covers the DMA prologue, plus PE warmup matmuls to hold
the HAM clock-gate at 2.4GHz.
Host side: transpose x to [B, D, S] bf16 with zero padding baked in,
pre-arrange weights as [D, tap, c] bf16, linear weights fp16, and transpose
the [B, NOUT, S] device output back to [B, S, NOUT].
"""

import sys

sys.path.insert(0, "/opt/trn_rl_repo")

import ml_dtypes
import numpy as np

import concourse.tile as tile
import concourse.mybir as mybir
from concourse import bacc
from concourse.bass_utils import run_bass_kernel_spmd

B, S, D = 32, 512, 512
NK = 256
KS = (3, 5, 7)
NOUT = 64
NCORES = 8
BPC = B // NCORES  # batches per core
PAD = 3  # (max(KS) - 1) // 2, baked into the padded x layout
SP = S + 2 * PAD
NTAP = sum(KS)  # 15
CIN = NK * len(KS)  # 768
NCT = CIN // 128  # 6 channel tiles
F32 = mybir.dt.float32
F16 = mybir.dt.float16
BF = mybir.dt.bfloat16
E4 = mybir.dt.float8e4
DR = mybir.MatmulPerfMode.DoubleRow

# k7-branch taps computed in pure fp8 e4m3 via DoubleRow (256-row contraction
# per matmul, ~1.7x bf16 on HW): trades measured rel err 2.4e-3 -> 1.43e-2
# (gate 2e-2) for ~9us/core of PE time. Scales are exact powers of two
# (w*8, x/8) so fp8 products accumulate into the same PSUM as bf16 taps.
FP8_LOCAL = (4, 5, 6)  # local tap indices within the k=7 branch
FP8_GLOBAL = tuple(8 + t for t in FP8_LOCAL)

# (global tap index, x-offset within padded row) per branch
_TAPS = []
_g = 0
for _k in KS:
    _pk = (_k - 1) // 2
    _TAPS.append([(_g + _t, PAD - _pk + _t) for _t in range(_k)])
    _g += _k


def _build(reps=1):
    nc = bacc.Bacc("TRN2")
    x = nc.dram_tensor("x", [BPC, D, SP], BF, kind="ExternalInput").ap()
    w = nc.dram_tensor("w", [D, NTAP, NK], BF, kind="ExternalInput").ap()
    x8 = nc.dram_tensor("x8", [BPC, 2, 128, 2, SP], E4, kind="ExternalInput").ap()
    w8 = nc.dram_tensor(
        "w8", [2, 128, 2, len(FP8_LOCAL), NK], E4, kind="ExternalInput"
    ).ap()
    lw = nc.dram_tensor("lw", [CIN, NOUT], F16, kind="ExternalInput").ap()
    cb = nc.dram_tensor("cb", [128, NCT], F32, kind="ExternalInput").ap()
    lb = nc.dram_tensor("lb", [128, 1], F32, kind="ExternalInput").ap()
    out = nc.dram_tensor("o", [BPC, NOUT, S], F32, kind="ExternalOutput").ap()

    with tile.TileContext(nc) as tc:
        with (
            tc.tile_pool(name="wpool", bufs=1) as wpool,
            tc.tile_pool(name="cpool", bufs=1) as cpool,
            tc.tile_pool(name="xpool", bufs=1) as xpool,
            tc.tile_pool(name="fpool", bufs=1) as fpool,
            tc.tile_pool(name="pspool", bufs=6, space="PSUM") as pspool,
            tc.tile_pool(name="lpspool", bufs=2, space="PSUM") as lpspool,
            tc.tile_pool(name="opool", bufs=2) as opool,
        ):
            w_sb = [
                wpool.tile([128, NTAP, NK], BF, name=f"w_{d}") for d in range(4)
            ]
            lw_sb = [
                cpool.tile([128, NOUT], F16, name=f"lw_{j}") for j in range(NCT)
            ]
            cb_sb = cpool.tile([128, NCT], F32, name="cb")
            lb_sb = cpool.tile([128, 1], F32, name="lb")

            xb = {}
            for b in range(BPC):
                for d in range(4):
                    xb[(b, d)] = xpool.tile([128, SP], BF, name=f"x_b{b}_d{d}")
            x8_sb = {}
            for b in range(BPC):
                for blk in range(2):
                    x8_sb[(b, blk)] = xpool.tile([128, 2, SP], E4, name=f"x8_b{b}_k{blk}")
            w8_sb = [
                wpool.tile([128, 2, len(FP8_LOCAL), NK], E4, name=f"w8_{blk}")
                for blk in range(2)
            ]

            # tiny bias tensors first: every relu needs cb
            nc.scalar.dma_start(cb_sb[:], cb[:, :])
            nc.scalar.dma_start(lb_sb[:], lb[:, :])
            # batch-0 x leads on the ACT queue; weights stream branch-ordered
            # (k3 first) on the SP queue so the first groups start ~1us in
            for d in range(4):
                nc.scalar.dma_start(xb[(0, d)][:], x[0, d * 128 : (d + 1) * 128, :])
            t0 = 0
            for ki, k in enumerate(KS):
                for d in range(4):
                    nc.sync.dma_start(
                        w_sb[d][:, t0 : t0 + k, :],
                        w[d * 128 : (d + 1) * 128, t0 : t0 + k, :],
                    )
                t0 += k
                if ki == 0:
                    for b in range(1, BPC):
                        for d in range(4):
                            nc.scalar.dma_start(
                                xb[(b, d)][:], x[b, d * 128 : (d + 1) * 128, :]
                            )
            for j in range(NCT):
                nc.sync.dma_start(lw_sb[j][:], lw[j * 128 : (j + 1) * 128, :])
            # fp8 operands for the k7 DoubleRow taps on the otherwise-idle
            # gpsimd queue; first needed ~2/3 into the first workload
            for blk in range(2):
                nc.gpsimd.dma_start(w8_sb[blk][:], w8[blk])
            for b in range(BPC):
                for blk in range(2):
                    nc.gpsimd.dma_start(x8_sb[(b, blk)][:], x8[b, blk])

            # PE warmup: dummy matmuls on a zeroed tile during the DMA
            # prologue keep the HAM clock-gate warm
            wm = cpool.tile([128, 640], BF, name="wm")
            nc.vector.memset(wm[:].bitcast(F32), 0.0)
            for g in range(2):
                wps = pspool.tile([128, S], F32, tag="ps", name=f"warm_ps_{g}")
                for i in range(4):
                    nc.tensor.matmul(
                        wps[:],
                        wm[:, 0:128],
                        wm[:, 128:640],
                        start=(i == 0),
                        stop=(i == 3),
                    )

            def one_workload(rep):
                feats = {}

                def linear(b):
                    lps = lpspool.tile([NOUT, S], F32, tag="lps", name=f"lps_r{rep}_b{b}")
                    for j in range(NCT):
                        nc.tensor.matmul(
                            lps[:],
                            lw_sb[j][:],
                            feats[(b, j)][:],
                            start=(j == 0),
                            stop=(j == NCT - 1),
                        )
                    osb = opool.tile([NOUT, S], F32, tag="osb", name=f"o_r{rep}_b{b}")
                    nc.vector.tensor_scalar_add(osb[:], lps[:], lb_sb[0:NOUT, 0:1])
                    nc.scalar.dma_start(out[b], osb[:])

                # branch-outer / batch-inner: all batches' k3 groups first, so
                # early compute covers the k5/k7 weight stream; each batch's
                # linear layer runs right after its last k7 group
                gi = 0
                for br, taps in enumerate(_TAPS):
                    for b in range(BPC):
                        for ct in range(2):
                            j = br * 2 + ct
                            ps = pspool.tile(
                                [128, S], F32, tag="ps", name=f"ps_r{rep}_b{b}_j{j}"
                            )
                            bf_taps = [t for t in taps if t[0] not in FP8_GLOBAL]
                            f8_taps = [t for t in taps if t[0] in FP8_GLOBAL]
                            nmm = len(bf_taps) * 4 + len(f8_taps) * 2
                            cs = slice(ct * 128, (ct + 1) * 128)
                            i = 0
                            # d-outer: each arriving weight d-chunk unlocks a
                            # run of taps, smoothing the prologue dribble
                            for d in range(4):
                                for tap, off in bf_taps:
                                    nc.tensor.matmul(
                                        ps[:],
                                        w_sb[d][:, tap, cs],
                                        xb[(b, d)][:, off : off + S],
                                        start=(i == 0),
                                        stop=(i == nmm - 1),
                                    )
                                    i += 1
                            for blk in range(2):
                                for tap, off in f8_taps:
                                    ti = FP8_GLOBAL.index(tap)
                                    nc.tensor.matmul(
                                        ps[:],
                                        w8_sb[blk][:, :, ti, cs],
                                        x8_sb[(b, blk)][:, :, off : off + S],
                                        start=(i == 0),
                                        stop=(i == nmm - 1),
                                        perf_mode=DR,
                                    )
                                    i += 1
                            f = fpool.tile(
                                [128, S], F16, name=f"f_r{rep}_b{b}_j{j}", tag=f"f_b{b}_j{j}"
                            )
                            # f = max(ps + bias, 0) in fp16; alternate DVE and
                            # ACT so the PSUM drain never gates the PE
                            if gi % 2 == 0:
                                nc.vector.tensor_scalar(
                                    f[:],
                                    ps[:],
                                    cb_sb[:, j : j + 1],
                                    0.0,
                                    mybir.AluOpType.add,
                                    mybir.AluOpType.max,
                                )
                            else:
                                nc.scalar.activation(
                                    f[:],
                                    ps[:],
                                    mybir.ActivationFunctionType.Relu,
                                    bias=cb_sb[:, j : j + 1],
                                )
                            gi += 1
                            feats[(b, j)] = f
                            if br == len(_TAPS) - 1 and ct == 1:
                                linear(b)

            if reps == 1:
                one_workload(0)
            else:
                with tc.For_i(0, reps, 1, hint_engines=(mybir.EngineType.PE,)):
                    one_workload(0)

    nc.compile()
    return nc


def _prep_inputs(x, conv_w3, conv_b3, conv_w5, conv_b5, conv_w7, conv_b7, lin_w, lin_b):
    x = np.asarray(x, np.float32)
    xt = np.zeros((B, D, SP), np.float32)
    xt[:, :, PAD : PAD + S] = x.transpose(0, 2, 1)
    xp = xt.astype(ml_dtypes.bfloat16)
    # fp8 copy of x at 1/8 scale in DoubleRow layout:
    # [B, D, SP] -> [B, blk, plane, part, SP] -> [B, blk, part, plane, SP]
    x8 = np.ascontiguousarray(
        (xt * np.float32(0.125))
        .astype(ml_dtypes.float8_e4m3)
        .reshape(B, 2, 2, 128, SP)
        .transpose(0, 1, 3, 2, 4)
    )
    # W[d, tap, c] = conv_wk[c, d, t], taps stacked k3|k5|k7
    Wf = np.concatenate(
        [
            np.asarray(cw, np.float32).transpose(1, 2, 0)
            for cw in (conv_w3, conv_w5, conv_w7)
        ],
        axis=1,
    )
    W = np.ascontiguousarray(Wf.astype(ml_dtypes.bfloat16))
    # fp8 weights (x8 scale) for the DoubleRow taps, same d decomposition
    w8 = np.ascontiguousarray(
        (Wf[:, list(FP8_GLOBAL), :] * np.float32(8.0))
        .astype(ml_dtypes.float8_e4m3)
        .reshape(2, 2, 128, len(FP8_LOCAL), NK)
        .transpose(0, 2, 1, 3, 4)
    )
    lwT = np.ascontiguousarray(np.asarray(lin_w, np.float32).T.astype(np.float16))
    cbT = np.ascontiguousarray(
        np.concatenate(
            [np.asarray(b_, np.float32) for b_ in (conv_b3, conv_b5, conv_b7)]
        ).reshape(NCT, 128).T
    )
    lb1 = np.asarray(lin_b, np.float32).reshape(NOUT, 1)
    lb2 = np.ascontiguousarray(np.concatenate([lb1, lb1], axis=0))
    return [
        {
            "x": np.ascontiguousarray(xp[c * BPC : (c + 1) * BPC]),
            "x8": np.ascontiguousarray(x8[c * BPC : (c + 1) * BPC]),
            "w": W,
            "w8": w8,
            "lw": lwT,
            "cb": cbT,
            "lb": lb2,
        }
        for c in range(NCORES)
    ]


_NC_CACHE = {}


def _get_nc(reps=1):
    if reps not in _NC_CACHE:
        _NC_CACHE[reps] = _build(reps)
    return _NC_CACHE[reps]


def kernel(x, conv_w3, conv_b3, conv_w5, conv_b5, conv_w7, conv_b7, lin_w, lin_b):
    nc = _get_nc(1)
    in_maps = _prep_inputs(
        x, conv_w3, conv_b3, conv_w5, conv_b5, conv_w7, conv_b7, lin_w, lin_b
    )
    res = run_bass_kernel_spmd(nc, in_maps, list(range(NCORES)))
    outT = np.concatenate([res.results[c]["o"] for c in range(NCORES)], axis=0)
    return np.ascontiguousarray(outT.transpose(0, 2, 1))
